# revision 1
# baseline (speedup 1.0000x reference)
"""DeepSeekMoE Trainium2 kernel (8 NeuronCores, data-parallel over tokens).

Problem: B=4, S=8192, H=576, I=512, E=8 routed experts (top-2) + 1 shared.
  y = shared_mlp(x) + sum_e w_e * expert_e_mlp(x),  w = renormalized top-2
  softmax router weights (dense-equivalent: non-selected experts get w=0).

Strategy:
  - Data-parallel: 32768 tokens split 4096/core across 8 cores.
  - Host pre-transposes x to [H, T] layout (H on partitions), padded
    H 576->640 (=5*128); row 576 is set to 1.0 so the router bias folds
    into the router matmul as an extra contraction row.
  - Router runs in exact fp32 (top-2 selection margins are as small as
    7.7e-6 in logit space); expert matmuls run in float32r (full PE rate
    at free-dim 512, ~1.6e-4 scale-relative error).
  - Top-2 on logits: m1/m2 via free-dim reduce_max + masking; renormalized
    weights via sigmoid(m1-m2) (softmax denominator cancels in the
    renormalization). Per-token weight rows are transposed via the PE and
    broadcast across partitions with a one-hot-selector matmul.
  - Per (token-block, expert): SwiGLU with PSUM accumulation over K-tiles;
    the routing weight is folded into h = silu(g)*u*w before the down
    matmul; y accumulates in SBUF across experts (shared expert
    initializes y). Tokens are processed in 4 blocks of 1024 with
    double-buffered x/y tiles so block boundaries overlap; expert weights
    stream per block, double-buffered.
"""
import numpy as np

NCORES = 8
B, S, H = 4, 8192, 576
I = 512
E = 8
T = B * S                 # 32768
TL = T // NCORES          # 4096 tokens per core
HP = 640                  # padded H (5*128); row 576 = bias row for router
KH = HP // 128            # 5 contraction tiles over H
KI = I // 128             # 4 contraction tiles over I
HT = HP // 128            # 5 output tiles over H
IT = I // 128             # 4 output tiles over I
CH = 512                  # token chunk (one PSUM bank at fp32)
NCH = TL // CH            # 8 chunks per core
NHALF = 4                 # token blocks per core (weight streaming granularity)
HALF = TL // NHALF        # 2048
RC = TL // 128            # 32 router chunks

_SILU_SUB_SIGMOID = False  # CoreSim has no Silu LUT; tests substitute Sigmoid

_cached = {}


def _build_program(repeat=1):
    import concourse.tile as tile
    from concourse import bacc, mybir
    from concourse.masks import make_identity
    from contextlib import ExitStack

    f32 = mybir.dt.float32
    f32r = mybir.dt.float32r

    nc = bacc.Bacc("TRN2", target_bir_lowering=False, debug=False,
                   num_devices=NCORES)

    xtf_d = nc.dram_tensor("xtf", [128, KH, TL], f32, kind="ExternalInput").ap()
    xtr_d = nc.dram_tensor("xtr", [128, KH, TL], f32r, kind="ExternalInput").ap()
    wg_d = nc.dram_tensor("wgall", [E + 1, 128, KH, I], f32r, kind="ExternalInput").ap()
    wu_d = nc.dram_tensor("wuall", [E + 1, 128, KH, I], f32r, kind="ExternalInput").ap()
    wd_d = nc.dram_tensor("wdall", [E + 1, 128, KI, HP], f32r, kind="ExternalInput").ap()
    wr_d = nc.dram_tensor("wrf", [128, KH, E], f32, kind="ExternalInput").ap()
    sel_d = nc.dram_tensor("selmat", [E, E, 128], f32r, kind="ExternalInput").ap()
    yt_d = nc.dram_tensor("yt", [128, HT, TL], f32, kind="ExternalOutput").ap()

    RCB = HALF // 128          # router chunks per token block

    with tile.TileContext(nc) as tc, ExitStack() as ctx:
        const = ctx.enter_context(tc.tile_pool(name="const", bufs=1))
        lpool = ctx.enter_context(tc.tile_pool(name="l", bufs=2))
        rpool = ctx.enter_context(tc.tile_pool(name="router", bufs=3))
        xpool = ctx.enter_context(tc.tile_pool(name="x", bufs=2))
        ypool = ctx.enter_context(tc.tile_pool(name="y", bufs=2))
        wpool = ctx.enter_context(tc.tile_pool(name="w", bufs=2))
        hpool = ctx.enter_context(tc.tile_pool(name="h", bufs=2))
        spool = ctx.enter_context(tc.tile_pool(name="s", bufs=2))
        psum = ctx.enter_context(tc.tile_pool(name="ps", bufs=1, space="PSUM"))

        # ---- constants
        wr_s = const.tile([128, KH, E], f32)
        nc.sync.dma_start(wr_s[:], wr_d[:])
        sel_s = const.tile([E, E, 128], f32r)
        nc.sync.dma_start(sel_s[:], sel_d[:])
        ident = const.tile([128, 128], f32)
        make_identity(nc, ident[:])

        def expert_block(e, xr, y, wt_sb):
            """One expert over one token block. e==0 is the shared expert
            (no routing weight, initializes y)."""
            wg = wpool.tile([128, KH, I], f32r, tag="wg")
            nc.sync.dma_start(wg[:], wg_d[e])
            wu = wpool.tile([128, KH, I], f32r, tag="wu")
            nc.sync.dma_start(wu[:], wu_d[e])
            wd = wpool.tile([128, KI, HP], f32r, tag="wd")
            nc.sync.dma_start(wd[:], wd_d[e])
            for c in range(HALF // CH):
                tok = slice(c * CH, (c + 1) * CH)
                if e > 0:
                    wb = psum.tile([128, CH], f32, name="wb")
                    nc.tensor.matmul(
                        wb[:], sel_s[:, e - 1],
                        wt_sb[:, c * (CH // 128):(c + 1) * (CH // 128)],
                        start=True, stop=True)
                h = hpool.tile([128, IT, CH], f32r, tag="h")
                for i in range(IT):
                    g_ps = psum.tile([128, CH], f32, name="g")
                    for k in range(KH):
                        nc.tensor.matmul(g_ps[:], wg[:, k, i * 128:(i + 1) * 128],
                                         xr[:, k, tok],
                                         start=(k == 0), stop=(k == KH - 1))
                    u_ps = psum.tile([128, CH], f32, name="u")
                    for k in range(KH):
                        nc.tensor.matmul(u_ps[:], wu[:, k, i * 128:(i + 1) * 128],
                                         xr[:, k, tok],
                                         start=(k == 0), stop=(k == KH - 1))
                    sg = spool.tile([128, CH], f32, tag="sg")
                    act = (mybir.ActivationFunctionType.Sigmoid
                           if _SILU_SUB_SIGMOID
                           else mybir.ActivationFunctionType.Silu)
                    nc.scalar.activation(sg[:], g_ps[:], act)
                    if e == 0:
                        nc.vector.tensor_tensor(h[:, i], sg[:], u_ps[:],
                                                mybir.AluOpType.mult)
                    else:
                        hx = spool.tile([128, CH], f32, tag="hx")
                        nc.vector.tensor_tensor(hx[:], sg[:], u_ps[:],
                                                mybir.AluOpType.mult)
                        nc.vector.tensor_tensor(h[:, i], hx[:], wb[:],
                                                mybir.AluOpType.mult)
                for j in range(HT):
                    yd = psum.tile([128, CH], f32, name=f"yd{j}")
                    for i in range(IT):
                        nc.tensor.matmul(yd[:], wd[:, i, j * 128:(j + 1) * 128],
                                         h[:, i],
                                         start=(i == 0), stop=(i == IT - 1))
                    if e == 0:
                        nc.vector.tensor_copy(y[:, j, tok], yd[:])
                    else:
                        nc.vector.tensor_tensor(y[:, j, tok], y[:, j, tok],
                                                yd[:], mybir.AluOpType.add)

        for _rep in range(repeat):
            for blk in range(NHALF):
                xr = xpool.tile([128, KH, HALF], f32r, tag="xr")
                for c in range(HALF // CH):   # piecewise: compute starts
                    nc.sync.dma_start(        # after the first chunk lands
                        xr[:, :, c * CH:(c + 1) * CH],
                        xtr_d[:, :, blk * HALF + c * CH:blk * HALF + (c + 1) * CH])
                y = ypool.tile([128, HT, HALF], f32, tag="y")

                # shared expert first: no routing dependency, keeps the PE
                # busy while the router/widget below runs in its shadow
                expert_block(0, xr, y, None)

                # ---- router for this block: exact fp32 matmuls reading the
                # fp32r x tile via bitcast (same raw fp32 bits; the DMA never
                # rounds). PSUM aliases the "wb" bank, idle in this window.
                logits = lpool.tile([128, RCB, E], f32, tag="logits")
                for c in range(RCB):
                    xf = rpool.tile([128, KH, 128], f32, tag="xf")
                    nc.sync.dma_start(
                        xf[:], xtf_d[:, :, blk * HALF + c * 128:
                                      blk * HALF + (c + 1) * 128])
                    lg = psum.tile([128, E], f32, name="wb")
                    for k in range(KH):
                        nc.tensor.matmul(lg[:], xf[:, k], wr_s[:, k],
                                         start=(k == 0), stop=(k == KH - 1))
                    nc.vector.tensor_copy(logits[:, c], lg[:])

                shp = [128, RCB, E]
                m1 = const.tile([128, RCB, 1], f32)
                nc.vector.tensor_reduce(m1[:], logits[:], mybir.AxisListType.X,
                                        mybir.AluOpType.max)
                nlt = const.tile(shp, f32, tag="wsA")   # 1.0 where logit < m1
                nc.vector.tensor_tensor(nlt[:], logits[:], m1[:].to_broadcast(shp),
                                        mybir.AluOpType.is_lt)
                t1 = const.tile(shp, f32, tag="wsB")    # below-max logits, else -1e30
                nc.vector.tensor_tensor(t1[:], logits[:], nlt[:], mybir.AluOpType.mult)
                t2 = const.tile(shp, f32, tag="wsC")
                nc.vector.tensor_scalar(t2[:], nlt[:], 1e30, -1e30,
                                        mybir.AluOpType.mult, mybir.AluOpType.add)
                nc.vector.tensor_tensor(t1[:], t1[:], t2[:], mybir.AluOpType.add)
                m2 = const.tile([128, RCB, 1], f32)
                nc.vector.tensor_reduce(m2[:], t1[:], mybir.AxisListType.X,
                                        mybir.AluOpType.max)
                d12 = const.tile([128, RCB, 1], f32)
                nc.vector.tensor_tensor(d12[:], m1[:], m2[:], mybir.AluOpType.subtract)
                whi = const.tile([128, RCB, 1], f32)
                nc.scalar.activation(whi[:], d12[:],
                                     mybir.ActivationFunctionType.Sigmoid)
                wlo = const.tile([128, RCB, 1], f32)
                nc.vector.tensor_scalar(wlo[:], whi[:], -1.0, 1.0,
                                        mybir.AluOpType.mult, mybir.AluOpType.add)
                mask1 = const.tile(shp, f32, tag="wsC")      # t2 dead
                nc.vector.tensor_tensor(mask1[:], logits[:], m1[:].to_broadcast(shp),
                                        mybir.AluOpType.is_ge)
                selm = const.tile(shp, f32, tag="wsA")       # nlt dead
                nc.vector.tensor_tensor(selm[:], logits[:], m2[:].to_broadcast(shp),
                                        mybir.AluOpType.is_ge)
                mask2 = const.tile(shp, f32, tag="wsB")      # t1 dead
                nc.vector.tensor_tensor(mask2[:], selm[:], mask1[:],
                                        mybir.AluOpType.subtract)
                wd1 = const.tile(shp, f32, tag="wsA")        # selm dead
                nc.vector.tensor_tensor(wd1[:], mask1[:], whi[:].to_broadcast(shp),
                                        mybir.AluOpType.mult)
                wd2 = const.tile(shp, f32, tag="wsC")        # mask1 dead
                nc.vector.tensor_tensor(wd2[:], mask2[:], wlo[:].to_broadcast(shp),
                                        mybir.AluOpType.mult)
                wdense = const.tile(shp, f32, tag="wsB")     # mask2 dead
                nc.vector.tensor_tensor(wdense[:], wd1[:], wd2[:],
                                        mybir.AluOpType.add)

                wt_sb = lpool.tile([E, RCB, 128], f32r, tag="wt")
                for c in range(RCB):
                    tp = psum.tile([E, 128], f32, name="wb")
                    nc.tensor.transpose(tp[:], wdense[:, c], ident[:])
                    nc.vector.tensor_copy(wt_sb[:, c], tp[:])

                for e in range(1, E + 1):
                    expert_block(e, xr, y, wt_sb)

                nc.sync.dma_start(yt_d[:, :, blk * HALF:(blk + 1) * HALF], y[:])

    nc.compile()
    return nc


def _get_program():
    if "nc" not in _cached:
        _cached["nc"] = _build_program()
    return _cached["nc"]


def _shard_inputs(x, shared_gate_w, shared_up_w, shared_down_w,
                  routed_gate_w, routed_up_w, routed_down_w,
                  router_w, router_bias):
    """Build the 8 per-core input maps (host-side layout prep only)."""
    f = np.float32
    xf = np.ascontiguousarray(np.asarray(x, f).reshape(T, H))

    def stack_kxm(shared, routed, kdim, mdim):
        # [9, kdim(=H), mdim] zero-padded on K to KH*128, tiled to
        # [9, 128, ktiles, mdim]
        w = np.concatenate([np.asarray(shared, f)[None],
                            np.asarray(routed, f)], axis=0)
        kp = ((kdim + 127) // 128) * 128
        out = np.zeros((E + 1, kp, mdim), f)
        out[:, :kdim] = w
        kt = kp // 128
        return np.ascontiguousarray(
            out.reshape(E + 1, kt, 128, mdim).transpose(0, 2, 1, 3))

    wgall = stack_kxm(shared_gate_w, routed_gate_w, H, I)
    wuall = stack_kxm(shared_up_w, routed_up_w, H, I)

    dn = np.concatenate([np.asarray(shared_down_w, f)[None],
                         np.asarray(routed_down_w, f)], axis=0)  # [9, I, H]
    dpad = np.zeros((E + 1, I, HP), f)
    dpad[:, :, :H] = dn
    wdall = np.ascontiguousarray(
        dpad.reshape(E + 1, KI, 128, HP).transpose(0, 2, 1, 3))

    rw = np.zeros((HP, E), f)
    rw[:H] = np.asarray(router_w, f)
    rw[H] = np.asarray(router_bias, f)          # bias row (x row 576 == 1.0)
    wrf = np.ascontiguousarray(rw.reshape(KH, 128, E).transpose(1, 0, 2))

    selmat = np.zeros((E, E, 128), f)
    for e in range(E):
        selmat[e, e, :] = 1.0

    in_maps = []
    for c in range(NCORES):
        xs = xf[c * TL:(c + 1) * TL]            # [TL, H]
        xp = np.zeros((TL, HP), f)
        xp[:, :H] = xs
        xp[:, H] = 1.0                          # bias row for the router
        xt = np.ascontiguousarray(
            xp.T.reshape(KH, 128, TL).transpose(1, 0, 2))
        in_maps.append({
            "xtf": xt, "xtr": xt,
            "wgall": wgall, "wuall": wuall, "wdall": wdall,
            "wrf": wrf, "selmat": selmat,
        })
    return in_maps


def _assemble_output(core_outs):
    y = np.empty((T, H), np.float32)
    for c in range(NCORES):
        yt = core_outs[c]["yt"]                 # [128, HT, TL]
        yh = yt.transpose(1, 0, 2).reshape(HP, TL)[:H]
        y[c * TL:(c + 1) * TL] = yh.T
    return y.reshape(B, S, H)


def kernel(**inputs):
    from concourse.bass_utils import run_bass_kernel_spmd
    nc = _get_program()
    in_maps = _shard_inputs(**inputs)
    res = run_bass_kernel_spmd(nc, in_maps, list(range(NCORES)))
    return _assemble_output(res.results)



# revision 10
# speedup vs baseline: 2.9207x; 2.9207x over previous
"""DeepSeekMoE Trainium2 kernel (8 NeuronCores, expert-parallel dispatch).

Problem: B=4, S=8192, H=576, I=512, E=8 routed experts (top-2) + 1 shared.
  y = shared_mlp(x) + sum_e w_e * expert_e_mlp(x),  w = renormalized top-2
  softmax router weights. Non-selected experts have w == 0 exactly, so the
  sparse (routed) computation equals the reference's dense formulation up to
  fp rounding: only shared + 2 selected experts per token.

Strategy:
  - Host computes the (tiny, 151M-MAC) router and dispatches: routed expert
    e's tokens all go to core e (expert-parallel); every core also runs the
    shared expert over its 4096 resident tokens. Per-core slots:
    4096 shared + C_r routed (C_r = max_e count_e rounded to 128, ~8448)
    = ~12544 slot-equivalents vs 36864 for the dense kernel (2.9x fewer).
  - The routing weight is folded into the up-projection input on the host
    (SwiGLU is linear in the up path: silu(x@Wg) * ((w*x)@Wu) @ Wd
    = w * expert(x)), so the device runs a pure uniform SwiGLU per chunk
    and the host scatter-ADD combines per-expert outputs.
  - bf16 operands (fp32 PSUM accumulate): enables the PE's automatic fast
    weight load (FWL) — fp32/fp32r weights disable FWL and leave ~107ns of
    LDWEIGHTS exposed per matmul. Measured headroom ~4e-3 rel err vs the
    2e-2 gate.
  - H=576 contraction = 4x128 + 64-row tail. The gate-tail and up-tail
    (both K=64, M=128) are packed into ONE PE pass via row-group tiling:
    combined weight tile rows 0:64 = gate tail, 64:128 = up tail, with the
    x tail duplicated into both partition halves. The M=64 down-tail
    output is packed via column-group tiling: i=0,2 accumulate into PSUM
    partitions 0:64, i=1,3 into 64:128, then one vector add merges halves.
    Per-chunk PE passes: 54 (vs 61 naive).
  - Each core keeps only TWO weight sets (shared + its one routed expert)
    SBUF-resident, loaded once. Per-iteration DMA is x in (bf16) + y out
    (fp32) ~54 MB, well under compute. x/y chunk tiles are double-buffered
    so DMA overlaps compute.
"""
import numpy as np
import ml_dtypes

BF16 = ml_dtypes.bfloat16

NCORES = 8
B, S, H = 4, 8192, 576
I = 512
E = 8
T = B * S                 # 32768
TL = T // NCORES          # 4096 resident tokens per core == shared slots
CH = 512                  # token chunk (one PSUM bank at fp32)
KH4 = 4                   # full 128-row contraction tiles over H
HTAIL = H - 4 * 128       # 64-row contraction/output tail
IT = I // 128             # 4 tiles over I
CS = TL                   # shared segment slots (exactly TL, no padding)

_SILU_SUB_SIGMOID = False  # CoreSim has no Silu LUT; tests substitute Sigmoid

_cached = {}
_plan = {}                 # set by _shard_inputs: routing/scatter info


def _build_program(repeat=1):
    import concourse.tile as tile
    from concourse import bacc, mybir
    from contextlib import ExitStack

    f32 = mybir.dt.float32
    bf16 = mybir.dt.bfloat16
    CR = _plan["CR"]
    SLOTS = CS + CR

    nc = bacc.Bacc("TRN2", target_bir_lowering=False, debug=False,
                   num_devices=NCORES)

    # x gate-path main rows [0:512); bf16
    xa_d = nc.dram_tensor("xa", [128, KH4, SLOTS], bf16, kind="ExternalInput").ap()
    # x up-path main rows, routing weight pre-folded; routed slots only
    xu_d = nc.dram_tensor("xu", [128, KH4, CR], bf16, kind="ExternalInput").ap()
    # tail rows [512:576) duplicated: rows 0:64 gate path, 64:128 up path
    xb_d = nc.dram_tensor("xb", [128, SLOTS], bf16, kind="ExternalInput").ap()
    # weight stacks: index 0 = shared expert, 1 = this core's routed expert
    wgm_d = nc.dram_tensor("wgm", [2, 128, KH4, I], bf16, kind="ExternalInput").ap()
    wum_d = nc.dram_tensor("wum", [2, 128, KH4, I], bf16, kind="ExternalInput").ap()
    # combined tails: rows 0:64 gate tail, 64:128 up tail
    wt2_d = nc.dram_tensor("wt2", [2, 128, I], bf16, kind="ExternalInput").ap()
    wdm_d = nc.dram_tensor("wdm", [2, 128, IT, H], bf16, kind="ExternalInput").ap()
    ya_d = nc.dram_tensor("ya", [128, KH4, SLOTS], f32, kind="ExternalOutput").ap()
    yb_d = nc.dram_tensor("yb", [HTAIL, SLOTS], f32, kind="ExternalOutput").ap()

    with tile.TileContext(nc) as tc, ExitStack() as ctx:
        const = ctx.enter_context(tc.tile_pool(name="const", bufs=1))
        xpool = ctx.enter_context(tc.tile_pool(name="x", bufs=3))
        ypool = ctx.enter_context(tc.tile_pool(name="y", bufs=3))
        hpool = ctx.enter_context(tc.tile_pool(name="h", bufs=2))
        spool = ctx.enter_context(tc.tile_pool(name="s", bufs=2))
        psum = ctx.enter_context(tc.tile_pool(name="ps", bufs=1, space="PSUM"))

        # ---- resident weights (loaded once; reused across repeats)
        def wload(nm, dram, shape):
            tiles = []
            for e in range(2):
                t = const.tile(shape, bf16, tag=f"{nm}{e}", name=f"{nm}{e}")
                nc.sync.dma_start(t[:], dram[e])
                tiles.append(t)
            return tiles

        wgm = wload("wgm", wgm_d, [128, KH4, I])
        wum = wload("wum", wum_d, [128, KH4, I])
        wt2 = wload("wt2", wt2_d, [128, I])
        wdm = wload("wdm", wdm_d, [128, IT, H])

        act = (mybir.ActivationFunctionType.Sigmoid if _SILU_SUB_SIGMOID
               else mybir.ActivationFunctionType.Silu)

        def issue_loads(e, off, n):
            """Queue the x DMAs for a chunk (one chunk ahead of compute, so
            loads sit before the previous chunk's y stores in the in-order
            SP queue)."""
            xa = xpool.tile([128, KH4, CH], bf16, tag="xa")
            nc.sync.dma_start(xa[:, :, :n], xa_d[:, :, off:off + n])
            xb = xpool.tile([128, CH], bf16, tag="xb")
            nc.sync.dma_start(xb[:, :n], xb_d[:, off:off + n])
            if e == 1:
                xu = xpool.tile([128, KH4, CH], bf16, tag="xu")
                nc.sync.dma_start(xu[:, :, :n], xu_d[:, :, off - CS:off - CS + n])
            else:
                xu = xa
            return xa, xb, xu

        def chunk(e, off, n, xa, xb, xu):
            """One SwiGLU chunk of n slots at slot offset off, expert e
            (0 = shared; 1 = routed, up-path input pre-scaled by routing w)."""
            h = hpool.tile([128, IT, CH], bf16, tag="h")
            for i in range(IT):
                mi = slice(i * 128, (i + 1) * 128)
                g_ps = psum.tile([128, CH], f32, name="g", bufs=2)
                u_ps = psum.tile([128, CH], f32, name="u", bufs=2)
                for k in range(KH4):
                    nc.tensor.matmul(g_ps[:, :n], wgm[e][:, k, mi],
                                     xa[:, k, :n], start=(k == 0), stop=False)
                for k in range(KH4):
                    nc.tensor.matmul(u_ps[:, :n], wum[e][:, k, mi],
                                     xu[:, k, :n], start=(k == 0), stop=False)
                # K=64 tails packed into one PE pass via row groups
                nc.tensor.matmul(g_ps[:, :n], wt2[e][0:64, mi], xb[0:64, :n],
                                 start=False, stop=True)
                nc.tensor.matmul(u_ps[:, :n], wt2[e][64:128, mi], xb[64:128, :n],
                                 start=False, stop=True, tile_position=(64, 0))
                sg = spool.tile([128, CH], f32, tag="sg")
                nc.scalar.activation(sg[:, :n], g_ps[:, :n], act)
                nc.vector.tensor_tensor(h[:, i, :n], sg[:, :n], u_ps[:, :n],
                                        mybir.AluOpType.mult)
            ya = ypool.tile([128, KH4, CH], f32, tag="ya")
            yb = ypool.tile([HTAIL, CH], f32, tag="yb")
            for j in range(KH4):
                yd = psum.tile([128, CH], f32, name=f"yd{j}")
                mj = slice(j * 128, (j + 1) * 128)
                for i in range(IT):
                    nc.tensor.matmul(yd[:, :n], wdm[e][:, i, mj], h[:, i, :n],
                                     start=(i == 0), stop=(i == IT - 1))
                nc.vector.tensor_copy(ya[:, j, :n], yd[:, :n])
            # M=64 down-tail packed via column groups: i=0,2 -> rows 0:64,
            # i=1,3 -> rows 64:128, then merge halves with one add.
            yd4 = psum.tile([128, CH], f32, name="yd0")  # reuses yd0 bank; its copy is long done
            mt = slice(4 * 128, 4 * 128 + HTAIL)
            nc.tensor.matmul(yd4[0:64, :n], wdm[e][:, 0, mt], h[:, 0, :n],
                             start=True, stop=False, tile_position=(0, 0))
            nc.tensor.matmul(yd4[64:128, :n], wdm[e][:, 1, mt], h[:, 1, :n],
                             start=True, stop=False, tile_position=(0, 64))
            nc.tensor.matmul(yd4[0:64, :n], wdm[e][:, 2, mt], h[:, 2, :n],
                             start=False, stop=True, tile_position=(0, 0))
            nc.tensor.matmul(yd4[64:128, :n], wdm[e][:, 3, mt], h[:, 3, :n],
                             start=False, stop=True, tile_position=(0, 64))
            # DVE can read only one operand from PSUM: stage one half through
            # the (otherwise idle) scalar engine into SBUF first.
            yt = spool.tile([HTAIL, CH], f32, tag="yt")
            nc.scalar.activation(yt[:, :n], yd4[0:64, :n],
                                 mybir.ActivationFunctionType.Copy)
            nc.vector.tensor_tensor(yb[:, :n], yt[:, :n], yd4[64:128, :n],
                                    mybir.AluOpType.add)
            nc.sync.dma_start(ya_d[:, :, off:off + n], ya[:, :, :n])
            nc.sync.dma_start(yb_d[:, off:off + n], yb[:, :n])

        chunks = []
        for _rep in range(repeat):
            for e, seg0, segn in ((0, 0, CS), (1, CS, CR)):
                off = seg0
                while off < seg0 + segn:
                    n = min(CH, seg0 + segn - off)
                    chunks.append((e, off, n))
                    off += n
        tiles_next = issue_loads(*chunks[0])
        for idx, ch in enumerate(chunks):
            tiles_cur = tiles_next
            if idx + 1 < len(chunks):
                tiles_next = issue_loads(*chunks[idx + 1])
            chunk(*ch, *tiles_cur)

    nc.compile()
    return nc


def _get_program():
    key = ("nc", _plan["CR"])
    if key not in _cached:
        _cached[key] = _build_program()
    return _cached[key]


def _route(x_flat, router_w, router_bias):
    """fp32 router identical to the reference: softmax, stable top-2,
    renormalize."""
    f = np.float32
    logits = x_flat @ np.asarray(router_w, f) + np.asarray(router_bias, f)
    lm = logits.max(axis=1, keepdims=True)
    p = np.exp(logits - lm, dtype=f)
    p = (p / p.sum(axis=1, keepdims=True)).astype(f)
    order = np.argsort(-p, axis=1, kind="stable")[:, :2]
    tw = np.take_along_axis(p, order, axis=1)
    tw = (tw / tw.sum(axis=1, keepdims=True)).astype(f)
    return order, tw


def _shard_inputs(x, shared_gate_w, shared_up_w, shared_down_w,
                  routed_gate_w, routed_up_w, routed_down_w,
                  router_w, router_bias):
    """Host-side dispatch: route, gather per-expert token blocks, build the
    8 per-core input maps. Sets the scatter plan used by _assemble_output."""
    f = np.float32
    xf = np.ascontiguousarray(np.asarray(x, f).reshape(T, H))
    order, tw = _route(xf, router_w, router_bias)

    toks, wts = [], []
    for e in range(E):
        sel = order == e                       # [T, 2]
        rows = np.where(sel.any(axis=1))[0]
        col = np.argmax(sel[rows], axis=1)
        toks.append(rows)
        wts.append(tw[rows, col].astype(f))
    counts = np.array([len(t) for t in toks])
    CR = int(((counts.max() + 127) // 128) * 128)
    _plan.clear()
    _plan.update({"CR": CR, "toks": toks, "counts": counts})
    SLOTS = CS + CR

    def ktile_main(w):                         # [H or I, M] -> [128, kt, M]
        k = (w.shape[0] // 128) * 128
        return np.ascontiguousarray(
            w[:k].reshape(-1, 128, w.shape[1]).transpose(1, 0, 2).astype(BF16))

    sg_w, su_w = np.asarray(shared_gate_w, f), np.asarray(shared_up_w, f)
    sd_w = np.asarray(shared_down_w, f)
    rg_w, ru_w = np.asarray(routed_gate_w, f), np.asarray(routed_up_w, f)
    rd_w = np.asarray(routed_down_w, f)

    def tails2(gw, uw):                        # [128, I]: gate tail ; up tail
        return np.concatenate([gw[512:], uw[512:]], axis=0).astype(BF16)

    in_maps = []
    for c in range(NCORES):
        nres = counts[c]
        w_res = wts[c]                         # [nres] routing weights
        xs = np.zeros((SLOTS, H), f)
        xs[:CS] = xf[c * TL:(c + 1) * TL]
        xs[CS:CS + nres] = xf[toks[c]]
        xsT = np.ascontiguousarray(xs.T)       # [H, SLOTS]
        xa = np.ascontiguousarray(
            xsT[:512].reshape(KH4, 128, SLOTS).transpose(1, 0, 2).astype(BF16))
        # up-path input for routed slots: x * routing weight
        xw = np.zeros((CR, H), f)
        xw[:nres] = xf[toks[c]] * w_res[:, None]
        xwT = np.ascontiguousarray(xw.T)
        xu = np.ascontiguousarray(
            xwT[:512].reshape(KH4, 128, CR).transpose(1, 0, 2).astype(BF16))
        # tails: rows 0:64 gate path (plain x), 64:128 up path (scaled on
        # routed slots, plain on shared slots)
        xb = np.empty((128, SLOTS), BF16)
        xb[0:64] = xsT[512:].astype(BF16)
        xb[64:128, :CS] = xsT[512:, :CS].astype(BF16)
        xb[64:128, CS:] = xwT[512:].astype(BF16)
        in_maps.append({
            "xa": xa, "xu": xu, "xb": xb,
            "wgm": np.stack([ktile_main(sg_w), ktile_main(rg_w[c])]),
            "wum": np.stack([ktile_main(su_w), ktile_main(ru_w[c])]),
            "wt2": np.stack([tails2(sg_w, su_w), tails2(rg_w[c], ru_w[c])]),
            "wdm": np.stack([ktile_main(sd_w), ktile_main(rd_w[c])]),
        })
    return in_maps


def _assemble_output(core_outs):
    y = np.zeros((T, H), np.float64)
    for c in range(NCORES):
        ya = core_outs[c]["ya"]                # [128, KH4, SLOTS]
        yb = core_outs[c]["yb"]                # [HTAIL, SLOTS]
        seg = np.concatenate(
            [ya.transpose(1, 0, 2).reshape(512, -1), yb], axis=0)  # [H, SLOTS]
        y[c * TL:(c + 1) * TL] = seg[:, :CS].T
    for c in range(NCORES):
        ya = core_outs[c]["ya"]
        yb = core_outs[c]["yb"]
        n = _plan["counts"][c]
        seg = np.concatenate(
            [ya.transpose(1, 0, 2).reshape(512, -1), yb],
            axis=0)[:, CS:CS + n]
        y[_plan["toks"][c]] += seg.T
    return y.astype(np.float32).reshape(B, S, H)


def kernel(**inputs):
    from concourse.bass_utils import run_bass_kernel_spmd
    in_maps = _shard_inputs(**inputs)
    nc = _get_program()
    res = run_bass_kernel_spmd(nc, in_maps, list(range(NCORES)))
    return _assemble_output(res.results)


# revision 12
# speedup vs baseline: 2.9283x; 1.0026x over previous
"""DeepSeekMoE Trainium2 kernel (8 NeuronCores, expert-parallel dispatch).

Problem: B=4, S=8192, H=576, I=512, E=8 routed experts (top-2) + 1 shared.
  y = shared_mlp(x) + sum_e w_e * expert_e_mlp(x),  w = renormalized top-2
  softmax router weights. Non-selected experts have w == 0 exactly, so the
  sparse (routed) computation equals the reference's dense formulation up to
  fp rounding: only shared + 2 selected experts per token.

Strategy:
  - Host computes the (tiny, 151M-MAC) router and dispatches: routed expert
    e's tokens all go to core e (expert-parallel); every core also runs the
    shared expert over its 4096 resident tokens. Per-core slots:
    4096 shared + C_r routed (C_r = max_e count_e rounded to 128, ~8448)
    = ~12544 slot-equivalents vs 36864 for the dense kernel (2.9x fewer).
  - The routing weight is folded into the up-projection input on the host
    (SwiGLU is linear in the up path: silu(x@Wg) * ((w*x)@Wu) @ Wd
    = w * expert(x)), so the device runs a pure uniform SwiGLU per chunk
    and the host scatter-ADD combines per-expert outputs.
  - bf16 operands (fp32 PSUM accumulate): enables the PE's automatic fast
    weight load (FWL) — fp32/fp32r weights disable FWL and leave ~107ns of
    LDWEIGHTS exposed per matmul. Measured headroom ~4e-3 rel err vs the
    2e-2 gate.
  - H=576 contraction = 4x128 + 64-row tail. The gate-tail and up-tail
    (both K=64, M=128) are packed into ONE PE pass via row-group tiling:
    combined weight tile rows 0:64 = gate tail, 64:128 = up tail, with the
    x tail duplicated into both partition halves. The M=64 down-tail
    output is packed via column-group tiling: i=0,2 accumulate into PSUM
    partitions 0:64, i=1,3 into 64:128, then one vector add merges halves.
    Per-chunk PE passes: 54 (vs 61 naive).
  - Each core keeps only TWO weight sets (shared + its one routed expert)
    SBUF-resident, loaded once. Per-iteration DMA is x in (bf16) + y out
    (fp32) ~54 MB, well under compute. x/y chunk tiles are double-buffered
    so DMA overlaps compute.
"""
import numpy as np
import ml_dtypes

BF16 = ml_dtypes.bfloat16

NCORES = 8
B, S, H = 4, 8192, 576
I = 512
E = 8
T = B * S                 # 32768
TL = T // NCORES          # 4096 resident tokens per core == shared slots
CH = 512                  # token chunk (one PSUM bank at fp32)
KH4 = 4                   # full 128-row contraction tiles over H
HTAIL = H - 4 * 128       # 64-row contraction/output tail
IT = I // 128             # 4 tiles over I
CS = TL                   # shared segment slots (exactly TL, no padding)

_SILU_SUB_SIGMOID = False  # CoreSim has no Silu LUT; tests substitute Sigmoid

_cached = {}
_plan = {}                 # set by _shard_inputs: routing/scatter info


def _build_program(repeat=1):
    import concourse.tile as tile
    from concourse import bacc, mybir
    from contextlib import ExitStack

    f32 = mybir.dt.float32
    bf16 = mybir.dt.bfloat16
    CR = _plan["CR"]
    SLOTS = CS + CR

    nc = bacc.Bacc("TRN2", target_bir_lowering=False, debug=False,
                   num_devices=NCORES)

    # x gate-path main rows [0:512); bf16
    xa_d = nc.dram_tensor("xa", [128, KH4, SLOTS], bf16, kind="ExternalInput").ap()
    # x up-path main rows, routing weight pre-folded; routed slots only
    xu_d = nc.dram_tensor("xu", [128, KH4, CR], bf16, kind="ExternalInput").ap()
    # tail rows [512:576) duplicated: rows 0:64 gate path, 64:128 up path
    xb_d = nc.dram_tensor("xb", [128, SLOTS], bf16, kind="ExternalInput").ap()
    # weight stacks: index 0 = shared expert, 1 = this core's routed expert
    wgm_d = nc.dram_tensor("wgm", [2, 128, KH4, I], bf16, kind="ExternalInput").ap()
    wum_d = nc.dram_tensor("wum", [2, 128, KH4, I], bf16, kind="ExternalInput").ap()
    # combined tails: rows 0:64 gate tail, 64:128 up tail
    wt2_d = nc.dram_tensor("wt2", [2, 128, I], bf16, kind="ExternalInput").ap()
    wdm_d = nc.dram_tensor("wdm", [2, 128, IT, H], bf16, kind="ExternalInput").ap()
    ya_d = nc.dram_tensor("ya", [128, KH4, SLOTS], f32, kind="ExternalOutput").ap()
    yb_d = nc.dram_tensor("yb", [HTAIL, SLOTS], f32, kind="ExternalOutput").ap()

    with tile.TileContext(nc) as tc, ExitStack() as ctx:
        const = ctx.enter_context(tc.tile_pool(name="const", bufs=1))
        xpool = ctx.enter_context(tc.tile_pool(name="x", bufs=3))
        ypool = ctx.enter_context(tc.tile_pool(name="y", bufs=3))
        hpool = ctx.enter_context(tc.tile_pool(name="h", bufs=2))
        spool = ctx.enter_context(tc.tile_pool(name="s", bufs=2))
        psum = ctx.enter_context(tc.tile_pool(name="ps", bufs=1, space="PSUM"))

        # ---- resident weights (loaded once; reused across repeats)
        def wload(nm, dram, shape):
            tiles = []
            for e in range(2):
                t = const.tile(shape, bf16, tag=f"{nm}{e}", name=f"{nm}{e}")
                nc.sync.dma_start(t[:], dram[e])
                tiles.append(t)
            return tiles

        wgm = wload("wgm", wgm_d, [128, KH4, I])
        wum = wload("wum", wum_d, [128, KH4, I])
        wt2 = wload("wt2", wt2_d, [128, I])
        wdm = wload("wdm", wdm_d, [128, IT, H])

        act = (mybir.ActivationFunctionType.Sigmoid if _SILU_SUB_SIGMOID
               else mybir.ActivationFunctionType.Silu)

        def issue_loads(e, off, n):
            """Queue the x DMAs for a chunk (one chunk ahead of compute, so
            loads sit before the previous chunk's y stores in the in-order
            SP queue)."""
            xa = xpool.tile([128, KH4, CH], bf16, tag="xa")
            nc.sync.dma_start(xa[:, :, :n], xa_d[:, :, off:off + n])
            xb = xpool.tile([128, CH], bf16, tag="xb")
            nc.sync.dma_start(xb[:, :n], xb_d[:, off:off + n])
            if e == 1:
                xu = xpool.tile([128, KH4, CH], bf16, tag="xu")
                nc.sync.dma_start(xu[:, :, :n], xu_d[:, :, off - CS:off - CS + n])
            else:
                xu = xa
            return xa, xb, xu

        def chunk(e, off, n, xa, xb, xu):
            """One SwiGLU chunk of n slots at slot offset off, expert e
            (0 = shared; 1 = routed, up-path input pre-scaled by routing w)."""
            h = hpool.tile([128, IT, CH], bf16, tag="h")
            for i in range(IT):
                mi = slice(i * 128, (i + 1) * 128)
                g_ps = psum.tile([128, CH], f32, name="g", bufs=2)
                u_ps = psum.tile([128, CH], f32, name="u", bufs=2)
                for k in range(KH4):
                    nc.tensor.matmul(g_ps[:, :n], wgm[e][:, k, mi],
                                     xa[:, k, :n], start=(k == 0), stop=False)
                for k in range(KH4):
                    nc.tensor.matmul(u_ps[:, :n], wum[e][:, k, mi],
                                     xu[:, k, :n], start=(k == 0), stop=False)
                # K=64 tails packed into one PE pass via row groups
                nc.tensor.matmul(g_ps[:, :n], wt2[e][0:64, mi], xb[0:64, :n],
                                 start=False, stop=True)
                nc.tensor.matmul(u_ps[:, :n], wt2[e][64:128, mi], xb[64:128, :n],
                                 start=False, stop=True, tile_position=(64, 0))
                sg = spool.tile([128, CH], f32, tag="sg")
                nc.scalar.activation(sg[:, :n], g_ps[:, :n], act)
                nc.vector.tensor_tensor(h[:, i, :n], sg[:, :n], u_ps[:, :n],
                                        mybir.AluOpType.mult)
            ya = ypool.tile([128, KH4, CH], f32, tag="ya")
            yb = ypool.tile([HTAIL, CH], f32, tag="yb")
            for j in range(KH4):
                yd = psum.tile([128, CH], f32, name=f"yd{j}")
                mj = slice(j * 128, (j + 1) * 128)
                for i in range(IT):
                    nc.tensor.matmul(yd[:, :n], wdm[e][:, i, mj], h[:, i, :n],
                                     start=(i == 0), stop=(i == IT - 1))
                nc.vector.tensor_copy(ya[:, j, :n], yd[:, :n])
            # M=64 down-tail packed via column groups: i=0,2 -> rows 0:64,
            # i=1,3 -> rows 64:128, then merge halves with one add.
            yd4 = psum.tile([128, CH], f32, name="yd0")  # reuses yd0 bank; its copy is long done
            mt = slice(4 * 128, 4 * 128 + HTAIL)
            nc.tensor.matmul(yd4[0:64, :n], wdm[e][:, 0, mt], h[:, 0, :n],
                             start=True, stop=False, tile_position=(0, 0))
            nc.tensor.matmul(yd4[64:128, :n], wdm[e][:, 1, mt], h[:, 1, :n],
                             start=True, stop=False, tile_position=(0, 64))
            nc.tensor.matmul(yd4[0:64, :n], wdm[e][:, 2, mt], h[:, 2, :n],
                             start=False, stop=True, tile_position=(0, 0))
            nc.tensor.matmul(yd4[64:128, :n], wdm[e][:, 3, mt], h[:, 3, :n],
                             start=False, stop=True, tile_position=(0, 64))
            # DVE can read only one operand from PSUM: stage one half through
            # the (otherwise idle) scalar engine into SBUF first.
            yt = spool.tile([HTAIL, CH], f32, tag="yt")
            nc.scalar.activation(yt[:, :n], yd4[0:64, :n],
                                 mybir.ActivationFunctionType.Copy)
            nc.vector.tensor_tensor(yb[:, :n], yt[:, :n], yd4[64:128, :n],
                                    mybir.AluOpType.add)
            nc.sync.dma_start(ya_d[:, :, off:off + n], ya[:, :, :n])
            nc.sync.dma_start(yb_d[:, off:off + n], yb[:, :n])

        chunks = []
        for _rep in range(repeat):
            for e, seg0, segn in ((0, 0, CS), (1, CS, CR)):
                off = seg0
                while off < seg0 + segn:
                    n = min(CH, seg0 + segn - off)
                    chunks.append((e, off, n))
                    off += n
        tiles_next = issue_loads(*chunks[0])
        for idx, ch in enumerate(chunks):
            tiles_cur = tiles_next
            if idx + 1 < len(chunks):
                tiles_next = issue_loads(*chunks[idx + 1])
            chunk(*ch, *tiles_cur)

    nc.compile()
    return nc


def _get_program():
    key = ("nc", _plan["CR"])
    if key not in _cached:
        _cached[key] = _build_program()
    return _cached[key]


def _route(x_flat, router_w, router_bias):
    """fp32 router identical to the reference: softmax, stable top-2,
    renormalize."""
    f = np.float32
    logits = x_flat @ np.asarray(router_w, f) + np.asarray(router_bias, f)
    lm = logits.max(axis=1, keepdims=True)
    p = np.exp(logits - lm, dtype=f)
    p = (p / p.sum(axis=1, keepdims=True)).astype(f)
    order = np.argsort(-p, axis=1, kind="stable")[:, :2]
    tw = np.take_along_axis(p, order, axis=1)
    tw = (tw / tw.sum(axis=1, keepdims=True)).astype(f)
    return order, tw


def _shard_inputs(x, shared_gate_w, shared_up_w, shared_down_w,
                  routed_gate_w, routed_up_w, routed_down_w,
                  router_w, router_bias):
    """Host-side dispatch: route, gather per-expert token blocks, build the
    8 per-core input maps. Sets the scatter plan used by _assemble_output."""
    f = np.float32
    xf = np.ascontiguousarray(np.asarray(x, f).reshape(T, H))
    order, tw = _route(xf, router_w, router_bias)

    toks, wts = [], []
    for e in range(E):
        sel = order == e                       # [T, 2]
        rows = np.where(sel.any(axis=1))[0]
        col = np.argmax(sel[rows], axis=1)
        toks.append(rows)
        wts.append(tw[rows, col].astype(f))
    counts = np.array([len(t) for t in toks])
    CR = int(((counts.max() + 127) // 128) * 128)
    _plan.clear()
    _plan.update({"CR": CR, "toks": toks, "counts": counts})
    SLOTS = CS + CR

    def ktile_main(w):                         # [H or I, M] -> [128, kt, M]
        k = (w.shape[0] // 128) * 128
        return np.ascontiguousarray(
            w[:k].reshape(-1, 128, w.shape[1]).transpose(1, 0, 2).astype(BF16))

    sg_w, su_w = np.asarray(shared_gate_w, f), np.asarray(shared_up_w, f)
    sd_w = np.asarray(shared_down_w, f)
    rg_w, ru_w = np.asarray(routed_gate_w, f), np.asarray(routed_up_w, f)
    rd_w = np.asarray(routed_down_w, f)

    def tails2(gw, uw):                        # [128, I]: gate tail ; up tail
        return np.concatenate([gw[512:], uw[512:]], axis=0).astype(BF16)

    in_maps = []
    for c in range(NCORES):
        nres = counts[c]
        w_res = wts[c]                         # [nres] routing weights
        xs = np.zeros((SLOTS, H), f)
        xs[:CS] = xf[c * TL:(c + 1) * TL]
        xs[CS:CS + nres] = xf[toks[c]]
        xsT = np.ascontiguousarray(xs.T)       # [H, SLOTS]
        xa = np.ascontiguousarray(
            xsT[:512].reshape(KH4, 128, SLOTS).transpose(1, 0, 2).astype(BF16))
        # up-path input for routed slots: x * routing weight
        xw = np.zeros((CR, H), f)
        xw[:nres] = xf[toks[c]] * w_res[:, None]
        xwT = np.ascontiguousarray(xw.T)
        xu = np.ascontiguousarray(
            xwT[:512].reshape(KH4, 128, CR).transpose(1, 0, 2).astype(BF16))
        # tails: rows 0:64 gate path (plain x), 64:128 up path (scaled on
        # routed slots, plain on shared slots)
        xb = np.empty((128, SLOTS), BF16)
        xb[0:64] = xsT[512:].astype(BF16)
        xb[64:128, :CS] = xsT[512:, :CS].astype(BF16)
        xb[64:128, CS:] = xwT[512:].astype(BF16)
        in_maps.append({
            "xa": xa, "xu": xu, "xb": xb,
            "wgm": np.stack([ktile_main(sg_w), ktile_main(rg_w[c])]),
            "wum": np.stack([ktile_main(su_w), ktile_main(ru_w[c])]),
            "wt2": np.stack([tails2(sg_w, su_w), tails2(rg_w[c], ru_w[c])]),
            "wdm": np.stack([ktile_main(sd_w), ktile_main(rd_w[c])]),
        })
    return in_maps


def _assemble_output(core_outs):
    y = np.zeros((T, H), np.float64)
    for c in range(NCORES):
        ya = core_outs[c]["ya"]                # [128, KH4, SLOTS]
        yb = core_outs[c]["yb"]                # [HTAIL, SLOTS]
        seg = np.concatenate(
            [ya.transpose(1, 0, 2).reshape(512, -1), yb], axis=0)  # [H, SLOTS]
        y[c * TL:(c + 1) * TL] = seg[:, :CS].T
    for c in range(NCORES):
        ya = core_outs[c]["ya"]
        yb = core_outs[c]["yb"]
        n = _plan["counts"][c]
        seg = np.concatenate(
            [ya.transpose(1, 0, 2).reshape(512, -1), yb],
            axis=0)[:, CS:CS + n]
        y[_plan["toks"][c]] += seg.T
    return y.astype(np.float32).reshape(B, S, H)


def kernel(**inputs):
    from concourse.bass_utils import run_bass_kernel_spmd
    in_maps = _shard_inputs(**inputs)
    nc = _get_program()
    res = run_bass_kernel_spmd(nc, in_maps, list(range(NCORES)))
    return _assemble_output(res.results)
